# revision 1
# baseline (speedup 1.0000x reference)
"""Trainium2 Bass kernel for nn_Attention_19104014533260.

Dense transformer attention block:
  qkv 1x1 conv + BN -> 4-head attention over 4096 pixels (d_k=32, d_v=64)
  -> + depthwise 3x3 conv(v) + BN -> proj 1x1 conv + BN.

Sharding: queries (pixel dim, n=4096) split across 8 cores; each core computes
all heads/batches for its 512-pixel slice. k/v are computed from the full
(replicated) x on every core, so there are no collectives.

Layout choices per core:
  - S^T[j, i] tiles [128 j-part, 512 i-free] from matmul(lhsT=k[:, jchunk],
    rhs=q_masked) with K=128 (q is masked per head so the other heads' rows
    contribute 0).
  - exp on ScalarE (PSUM -> SBUF, bf16 out) -- this is the bottleneck engine.
  - y = v @ P^T via bf16 matmuls accumulating over j-chunks; an appended
    ones-row in vT makes PSUM row 64 the softmax denominator for free.
  - depthwise 3x3 pe conv as 9 fused (v*w + acc) VectorE ops on a zero-padded
    v patch; BN biases folded on the host everywhere.
"""

import sys

sys.path.insert(0, "/opt/trn_rl_repo")

import numpy as np

import concourse.bass as bass
import concourse.bacc as bacc_mod
import concourse.tile as tile
from concourse import mybir

F32R = mybir.dt.float32r
F32 = mybir.dt.float32
BF16 = mybir.dt.bfloat16
F16 = mybir.dt.float16

EPS = 1e-5
NCORES = 8
N = 4096           # pixels per batch
SLICE = N // NCORES  # 512 query columns per core
B = 2
HEADS = 4
KD = 32            # key dim
HD = 64            # head (value) dim
DIM = 256

_CACHE = {}


def _build_program(debug=False):
    """Build (once) the single SPMD Bass program run on every core."""
    nc = bacc_mod.Bacc()

    x_d = nc.dram_tensor("x", [B, 2, 128, N], F32R, kind="ExternalInput")
    xr_d = nc.dram_tensor("xr", [B, 2, 128, 640], F32R, kind="ExternalInput")
    wk_d = nc.dram_tensor("wk", [128, 2, 128], F32R, kind="ExternalInput")
    wq_d = nc.dram_tensor("wq", [128, 2, 128], F32R, kind="ExternalInput")
    wva_d = nc.dram_tensor("wva", [128, 2, 260], F32R, kind="ExternalInput")
    wv_d = nc.dram_tensor("wv", [128, 2, 2, 128], F32R, kind="ExternalInput")
    wp_d = nc.dram_tensor("wp", [128, 2, 2, 128], F32R, kind="ExternalInput")
    bk_d = nc.dram_tensor("bk", [128, 1], F32, kind="ExternalInput")
    bq_d = nc.dram_tensor("bq", [128, 1], F32, kind="ExternalInput")
    bva_d = nc.dram_tensor("bva", [1, 260], F32, kind="ExternalInput")
    bv_d = nc.dram_tensor("bv", [128, 2], F32, kind="ExternalInput")
    bp_d = nc.dram_tensor("bp", [128, 2], F32, kind="ExternalInput")
    w9_d = nc.dram_tensor("w9", [128, 2, 9], F32, kind="ExternalInput")
    hmask_d = nc.dram_tensor("hmask", [128, 4], F32, kind="ExternalInput")
    vmask_d = nc.dram_tensor("vmask", [1, 640], F32, kind="ExternalInput")
    out_d = nc.dram_tensor("out", [B, 2, 128, SLICE], F32, kind="ExternalOutput")
    if debug:
        dk_d = nc.dram_tensor("dk", [128, 8, 512], F16, kind="ExternalOutput")
        dq_d = nc.dram_tensor("dq", [128, 4, 512], F16, kind="ExternalOutput")
        dva_d = nc.dram_tensor("dva", [128, 32, 260], F16, kind="ExternalOutput")
        dvp_d = nc.dram_tensor("dvp", [2, 128, 660], F32, kind="ExternalOutput")
        dz_d = nc.dram_tensor("dz", [2, 128, 512], F32, kind="ExternalOutput")
        dy_d = nc.dram_tensor("dy", [2, 128, 512], F32, kind="ExternalOutput")

    def bcast_rows(dram_ap, rows):
        # DRAM row tensor -> partition-replicated AP
        return bass.AP(
            tensor=dram_ap.tensor,
            offset=dram_ap.offset,
            ap=[[0, rows]] + [list(p) for p in dram_ap.ap[1:]],
        )

    with tile.TileContext(nc) as tc:
        with (
            tc.tile_pool(name="singles", bufs=1) as singles,
            tc.tile_pool(name="xp", bufs=8) as xp,
            tc.tile_pool(name="xrp", bufs=2) as xrp,
            tc.tile_pool(name="kp", bufs=16) as kp,
            tc.tile_pool(name="qp", bufs=4) as qp,
            tc.tile_pool(name="vap", bufs=64) as vap,
            tc.tile_pool(name="vpp", bufs=3) as vpp,
            tc.tile_pool(name="ep", bufs=8) as ep,
            tc.tile_pool(name="zp", bufs=8) as zp,
            tc.tile_pool(name="rp", bufs=4) as rp,
            tc.tile_pool(name="rfp", bufs=4) as rfp,
            tc.tile_pool(name="ytp", bufs=4) as ytp,
            tc.tile_pool(name="op", bufs=4) as op_,
            tc.tile_pool(name="drp", bufs=4, space="DRAM") as drp,
            tc.tile_pool(name="spool", bufs=2, space="PSUM") as spool,
            tc.tile_pool(name="ypool", bufs=2, space="PSUM") as ypool,
            tc.tile_pool(name="gpool", bufs=2, space="PSUM") as gpool,
        ):
            wk_s = singles.tile([128, 2, 128], F32R, tag="wk")
            wq_s = singles.tile([128, 2, 128], F32R, tag="wq")
            wva_s = singles.tile([128, 2, 260], F32R, tag="wva")
            wv_s = singles.tile([128, 2, 2, 128], F32R, tag="wv")
            wp_s = singles.tile([128, 2, 2, 128], F32R, tag="wp")
            bk_s = singles.tile([128, 1], F32, tag="bk")
            bq_s = singles.tile([128, 1], F32, tag="bq")
            bva_s = singles.tile([128, 260], F32, tag="bva")
            bv_s = singles.tile([128, 2], F32, tag="bv")
            bp_s = singles.tile([128, 2], F32, tag="bp")
            w9_s = singles.tile([128, 2, 9], F32, tag="w9")
            hmask_s = singles.tile([128, 4], F32, tag="hmask")
            vmask_s = singles.tile([128, 640], F32, tag="vmask")
            ones_s = singles.tile([1, 128], F32, tag="ones")
            nc.vector.memset(ones_s, 1.0)
            def load_weights_crit():
                # q/k-path weights ride the sync queue ahead of the x tiles
                for t, d in [(wq_s, wq_d), (wk_s, wk_d), (bq_s, bq_d),
                             (hmask_s, hmask_d), (bk_s, bk_d)]:
                    nc.sync.dma_start(t, d.ap())

            def load_weights_bulk():
                for t, d in [(wva_s, wva_d), (wv_s, wv_d), (bv_s, bv_d),
                             (wp_s, wp_d), (bp_s, bp_d), (w9_s, w9_d)]:
                    nc.gpsimd.dma_start(t, d.ap())
                nc.gpsimd.dma_start(bva_s, bcast_rows(bva_d.ap(), 128))
                nc.gpsimd.dma_start(vmask_s, bcast_rows(vmask_d.ap(), 128))

            TAPS = [(dy, dx) for dy in (-1, 0, 1) for dx in (-1, 0, 1)]

            state = {}

            def load_x(b):
                xrt = xrp.tile([128, 2, 640], F32R, tag="xr", name=f"xr{b}")
                nc.gpsimd.dma_start(
                    xrt, xr_d.ap()[b].rearrange("a p c -> p a c"))
                xts = []
                for n8 in range(8):
                    xt = xp.tile([128, 2, 512], F32R, tag="x", name=f"x{b}_{n8}")
                    nc.sync.dma_start(
                        xt, x_d.ap()[b, :, :, n8 * 512:(n8 + 1) * 512]
                        .rearrange("a p c -> p a c"))
                    xts.append(xt)
                state[b] = dict(xrt=xrt, xts=xts)

            def qkv(b):
                xrt = state[b]["xrt"]
                xts = state[b]["xts"]

                qts = [qp.tile([128, 2, 512], F16, tag="q", name=f"q{b}_{p}")
                       for p in range(2)]
                psq = gpool.tile([128, 512], F32, tag="g", name=f"qps{b}")
                for kc in range(2):
                    nc.tensor.matmul(psq, wq_s[:, kc, :], xrt[:, kc, 64:576],
                                     start=(kc == 0), stop=(kc == 1))
                def qcopy(h):
                    nc.vector.tensor_scalar(
                        out=qts[h // 2][:, h % 2, :], in0=psq,
                        scalar1=bq_s, scalar2=hmask_s[:, h:h + 1],
                        op0=mybir.AluOpType.add, op1=mybir.AluOpType.mult)

                def kchunk(n8):
                    ktc = kp.tile([128, 512], F16, tag="k", name=f"k{b}_{n8}")
                    ps = gpool.tile([128, 512], F32, tag="g", name=f"kps{b}_{n8}")
                    for kc in range(2):
                        nc.tensor.matmul(ps, wk_s[:, kc, :], xts[n8][:, kc, :],
                                         start=(kc == 0), stop=(kc == 1))
                    nc.vector.tensor_scalar_add(out=ktc, in0=ps, scalar1=bk_s)
                    return ktc

                # pair-0 q copies and the first k chunk lead so the first
                # S-matmul isn't queued behind all four q mask-copies on DVE
                qcopy(0)
                qcopy(1)
                kts = [kchunk(0)]
                qcopy(2)
                qcopy(3)
                for n8 in range(1, 8):
                    kts.append(kchunk(n8))

                vats = []
                for n32 in range(32):
                    vac = vap.tile([128, 260], F16, tag="va", name=f"va{b}_{n32}")
                    ps = gpool.tile([128, 260], F32, tag="g", name=f"vaps{b}_{n32}")
                    n8, sub = divmod(n32, 4)
                    for kc in range(2):
                        nc.tensor.matmul(
                            ps, xts[n8][:, kc, sub * 128:(sub + 1) * 128],
                            wva_s[:, kc, :], start=(kc == 0), stop=(kc == 1))
                    nc.vector.tensor_tensor(out=vac, in0=ps,
                                            in1=bva_s[:, :260],
                                            op=mybir.AluOpType.add)
                    vats.append(vac)

                vps = []
                for mc in range(2):
                    vpt = vpp.tile([128, 660], F32, tag="vp", name=f"vp{b}_{mc}")
                    nc.vector.memset(vpt, 0.0)
                    vpv = vpt.rearrange("p (r c) -> p r c", c=66)
                    for (c0, w, r0, nr) in [(0, 512, 0, 8), (512, 128, 8, 2)]:
                        ps = gpool.tile([128, w], F32, tag="g",
                                        name=f"vps{b}_{mc}_{c0}")
                        for kc in range(2):
                            nc.tensor.matmul(ps, wv_s[:, kc, mc, :],
                                             xrt[:, kc, c0:c0 + w],
                                             start=(kc == 0), stop=(kc == 1))
                        nc.vector.scalar_tensor_tensor(
                            out=vpv[:, r0:r0 + nr, 1:65],
                            in0=ps.rearrange("p (r c) -> p r c", c=64),
                            scalar=bv_s[:, mc:mc + 1],
                            in1=vmask_s[:, c0:c0 + w]
                            .rearrange("p (r c) -> p r c", c=64),
                            op0=mybir.AluOpType.add, op1=mybir.AluOpType.mult)
                    vps.append(vpt)
                state[b].update(kts=kts, qts=qts, vats=vats, vps=vps, zts={},
                                zys={})
                if debug and b == 0:
                    for n8 in range(8):
                        nc.sync.dma_start(dk_d.ap()[:, n8], kts[n8])
                    nc.sync.dma_start(dq_d.ap()[:, 0:2], qts[0])
                    nc.sync.dma_start(dq_d.ap()[:, 2:4], qts[1])
                    for n32 in range(32):
                        nc.sync.dma_start(dva_d.ap()[:, n32], vats[n32])
                    for mc in range(2):
                        nc.sync.dma_start(dvp_d.ap()[mc], vps[mc])

            def attn(b, ph):
                st = state[b]
                kts, vats = st["kts"], st["vats"]
                qt = st["qts"][ph]
                yts = [ypool.tile([128, 512], F32, tag="y", name=f"y{b}_{ph}_{hh}")
                       for hh in range(2)]
                ets = {}
                for jc in range(33):
                    if jc < 32:
                        sp = spool.tile([128, 1024], F32, tag="s",
                                        name=f"s{b}_{ph}_{jc}")
                        j8, jsub = divmod(jc, 4)
                        for hh in range(2):
                            h = 2 * ph + hh
                            nc.tensor.matmul(
                                sp[:, hh * 512:(hh + 1) * 512],
                                kts[j8][:, jsub * 128:(jsub + 1) * 128],
                                qt[:, hh, :], start=True, stop=True)
                        et = ep.tile([128, 1024], F16, tag="e",
                                     name=f"e{b}_{ph}_{jc}")
                        nc.scalar.activation(
                            et, sp, mybir.ActivationFunctionType.Exp)
                        ets[jc] = et
                    if jc >= 1:
                        et = ets.pop(jc - 1)
                        for hh in range(2):
                            h = 2 * ph + hh
                            nc.tensor.matmul(
                                yts[hh][0:65, :],
                                vats[jc - 1][:, 65 * h:65 * h + 65],
                                et[:, hh * 512:(hh + 1) * 512],
                                start=(jc == 1), stop=(jc == 32))
                yss = []
                for hh in range(2):
                    ys = ytp.tile([65, 512], F32, tag="yt", name=f"ys{b}_{ph}_{hh}")
                    nc.vector.tensor_copy(out=ys, in_=yts[hh][0:65, :])
                    yss.append(ys)
                st[f"y{ph}"] = yss

            def conv(b, ph):
                st = state[b]
                zt = zp.tile([128, 512], F32R, tag="z", name=f"z{b}_{ph}")
                zv = zt.rearrange("p (r c) -> p r c", c=64)
                vpv = st["vps"][ph].rearrange("p (r c) -> p r c", c=66)
                for t, (dy, dx) in enumerate(TAPS):
                    nc.vector.scalar_tensor_tensor(
                        out=zv,
                        in0=vpv[:, 1 + dy:9 + dy, 1 + dx:65 + dx],
                        scalar=w9_s[:, ph, t:t + 1], in1=zv,
                        op0=mybir.AluOpType.mult,
                        op1=(mybir.AluOpType.bypass if t == 0
                             else mybir.AluOpType.add))
                st["zts"][ph] = zt

            def norm(b, ph):
                st = state[b]
                yss = st[f"y{ph}"]
                zy = zp.tile([128, 512], F32R, tag="z", name=f"zy{b}_{ph}")
                for hh in range(2):
                    rt0 = rp.tile([1, 512], F32, tag="r0", name=f"r0_{b}{ph}{hh}")
                    # reciprocal_approx_fast misreads nonzero-base inputs;
                    # copy the sum row down to partition 0 first.
                    nc.vector.tensor_copy(out=rt0, in_=yss[hh][64:65, :])
                    rt = rp.tile([1, 512], F32, tag="r", name=f"r_{b}{ph}{hh}")
                    nc.vector.reciprocal_approx_fast(rt, rt0)
                    # broadcast r across partitions via a K=1 fp32 ones-matmul
                    # (faster than the DRAM round-trip and off the DMA queues)
                    rb = gpool.tile([128, 512], F32, tag="g",
                                    name=f"rb{b}{ph}{hh}")
                    nc.tensor.matmul(rb, ones_s, rt, start=True, stop=True)
                    # y*r goes straight into its own proj operand tile; the
                    # pe-conv z stays separate and proj accumulates both.
                    nc.vector.tensor_tensor(
                        out=zy[64 * hh:64 * hh + 64, :],
                        in0=yss[hh][0:64, :], in1=rb[0:64, :],
                        op=mybir.AluOpType.mult)
                st["zys"][ph] = zy
                if debug and b == 0:
                    nc.sync.dma_start(dy_d.ap()[ph, 0:65], yss[0])
                    nc.sync.dma_start(dz_d.ap()[ph], st["zts"][ph].bitcast(F32))

            def proj(b):
                st = state[b]
                for mo in range(2):
                    ps = gpool.tile([128, 512], F32, tag="g", name=f"pps{b}_{mo}")
                    for kc in range(2):
                        nc.tensor.matmul(ps, wp_s[:, kc, mo, :], st["zts"][kc],
                                         start=(kc == 0), stop=False)
                    for kc in range(2):
                        nc.tensor.matmul(ps, wp_s[:, kc, mo, :], st["zys"][kc],
                                         start=False, stop=(kc == 1))
                    ot = op_.tile([128, 512], F32, tag="o", name=f"o{b}_{mo}")
                    nc.vector.tensor_scalar_add(out=ot, in0=ps,
                                                scalar1=bp_s[:, mo:mo + 1])
                    nc.sync.dma_start(out_d.ap()[b, mo], ot)

            # cross-batch pipelined emission order: attention for the next
            # unit is emitted before the previous unit's finalize/proj so the
            # PE/ACT never drain at unit boundaries.
            load_weights_crit()
            load_x(0)
            load_weights_bulk()
            qkv(0)
            conv(0, 0)
            conv(0, 1)
            load_x(1)
            qkv(1)
            attn(0, 0)
            norm(0, 0)
            attn(0, 1)
            conv(1, 0)
            conv(1, 1)
            attn(1, 0)
            norm(0, 1)
            proj(0)
            attn(1, 1)
            norm(1, 0)
            norm(1, 1)
            proj(1)

    nc.compile()
    return nc


def _prep_maps(x, qkv_w, qkv_g, qkv_b, qkv_m, qkv_v,
               proj_w, proj_g, proj_b, proj_m, proj_v,
               pe_w, pe_g, pe_b, pe_m, pe_v):
    f = np.float32

    s_qkv = (qkv_g / np.sqrt(qkv_v + EPS)).astype(f)
    t_qkv = (qkv_b - qkv_m * s_qkv).astype(f)
    W = (qkv_w[:, :, 0, 0] * s_qkv[:, None]).astype(f)      # [512, 256]

    hs = np.arange(HEADS)
    qrows = (128 * hs[:, None] + np.arange(KD)[None, :]).ravel()
    krows = qrows + KD
    vrows = (128 * hs[:, None] + 2 * KD + np.arange(HD)[None, :]).ravel()

    scale = f(KD) ** -0.5
    wq_full = (W[qrows] * scale).astype(f)                  # [128, 256]
    bq_full = (t_qkv[qrows] * scale).astype(f)
    wk_full = W[krows]
    bk_full = t_qkv[krows]
    wv_full = W[vrows]                                      # [256, 256], vc=64h+d
    bv_full = t_qkv[vrows]

    def lhst_2(wfull):
        # [O=128, C=256] -> [c, kc, o] with c within 128-chunk kc
        return np.ascontiguousarray(
            wfull.T.reshape(2, 128, 128).transpose(1, 0, 2)).astype(f)

    wq_np = lhst_2(wq_full)
    wk_np = lhst_2(wk_full)

    V = np.zeros((DIM, 260), f)
    bva = np.zeros((1, 260), f)
    for h in range(HEADS):
        V[:, 65 * h:65 * h + 64] = wv_full[64 * h:64 * h + 64].T
        bva[0, 65 * h:65 * h + 64] = bv_full[64 * h:64 * h + 64]
        bva[0, 65 * h + 64] = 1.0
    wva_np = np.ascontiguousarray(
        V.reshape(2, 128, 260).transpose(1, 0, 2)).astype(f)

    def lhst_4(wfull):
        # [O=256, C=256] -> [c, kc, mo, o]
        return np.ascontiguousarray(
            wfull.reshape(2, 128, 2, 128).transpose(3, 2, 0, 1)).astype(f)

    wv_np = lhst_4(wv_full)
    bv_np = np.ascontiguousarray(bv_full.reshape(2, 128).T).astype(f)

    s_pe = (pe_g / np.sqrt(pe_v + EPS)).astype(f)
    t_pe = (pe_b - pe_m * s_pe).astype(f)
    w9_np = np.ascontiguousarray(
        (pe_w[:, 0].reshape(DIM, 9) * s_pe[:, None])
        .reshape(2, 128, 9).transpose(1, 0, 2)).astype(f)

    s_p = (proj_g / np.sqrt(proj_v + EPS)).astype(f)
    t_p = (proj_b - proj_m * s_p).astype(f)
    P_eff = (proj_w[:, :, 0, 0] * s_p[:, None]).astype(f)
    bp_full = (t_p + P_eff @ t_pe).astype(f)
    wp_np = lhst_4(P_eff)
    bp_np = np.ascontiguousarray(bp_full.reshape(2, 128).T).astype(f)

    hmask = np.zeros((128, 4), f)
    for h in range(HEADS):
        hmask[32 * h:32 * h + 32, h] = 1.0

    xf = np.ascontiguousarray(x.reshape(B, DIM, N)).astype(f)
    x_np = np.ascontiguousarray(xf.reshape(B, 2, 128, N))

    shared = dict(
        x=x_np, wk=wk_np, wq=wq_np, wva=wva_np, wv=wv_np, wp=wp_np,
        bk=np.ascontiguousarray(bk_full[:, None]),
        bq=np.ascontiguousarray(bq_full[:, None]),
        bva=bva, bv=bv_np, bp=bp_np, w9=w9_np, hmask=hmask,
    )

    in_maps = []
    for c in range(NCORES):
        own = SLICE * c
        lo, hi = own - 64, own + SLICE + 64
        xr = np.zeros((B, DIM, 640), f)
        a, bnd = max(lo, 0), min(hi, N)
        xr[:, :, a - lo:bnd - lo] = xf[:, :, a:bnd]
        vmask = np.ones((1, 640), f)
        if lo < 0:
            vmask[:, :-lo] = 0.0
        if hi > N:
            vmask[:, 640 - (hi - N):] = 0.0
        m = dict(shared)
        m["xr"] = np.ascontiguousarray(xr.reshape(B, 2, 128, 640))
        m["vmask"] = vmask
        in_maps.append(m)
    return in_maps


def _gather(results):
    full = np.zeros((B, DIM, N), np.float32)
    for c, res in enumerate(results):
        o = res["out"]  # [B, 2, 128, SLICE]
        for mo in range(2):
            full[:, 128 * mo:128 * (mo + 1), SLICE * c:SLICE * (c + 1)] = o[:, mo]
    return full.reshape(B, DIM, 64, 64)


def run(inputs, trace=False, trace_kwargs=None, debug=False):
    from concourse.bass_utils import run_bass_kernel_spmd
    key = ("nc", debug)
    if key not in _CACHE:
        _CACHE[key] = _build_program(debug)
    nc = _CACHE[key]
    in_maps = _prep_maps(**inputs)
    res = run_bass_kernel_spmd(
        nc, in_maps, core_ids=list(range(NCORES)),
        trace=trace, **(trace_kwargs or {}))
    return _gather(res.results), res


def kernel(**inputs):
    inputs = {k: np.asarray(v) for k, v in inputs.items()}
    out, _ = run(inputs, trace=False)
    return out



# revision 2
# speedup vs baseline: 1.0710x; 1.0710x over previous
"""Trainium2 Bass kernel for nn_Attention_19104014533260 (fp8 redesign).

Per core: own 512 query columns, full 4096 keys.
- S = x8^T p8 per head, contracting c=256 via fp8 DoubleRow (0.5 cyc/row);
  p8 = (AB*M) x8_own + AB*c with M = scale*Wk_h^T Wq_h folded on host, so
  there is no k tensor, no k-convert, and the q-side bias drops (it cancels
  in softmax). The k-side bias rides in via p's bias column c.
- exp(S - 2) (shift cancels in softmax, keeps e <= ~50 inside e4m3 range):
  split between ACT (exact exp, fp8 out) and DVE (Schraudolph: round to
  uint8 bits of e4m3, negatives saturate to +0).
- AV: out[65, 512] per (ph, hh) accumulating fp8 DoubleRow over j-pairs;
  vat row 0 is the ones column, so partition 0 of the PSUM is the softmax
  denominator (base-0 for reciprocal_approx_fast).
- pe depthwise 3x3 conv: 9 shifted bf16 matmuls over a host-padded 10x66
  window (wconv[t] = w9[:,t] * Wv) plus a rank-9 border matmul that adds
  bv*w9 only where taps are inside the image. BN biases and the bv-via-
  denominator correction are folded into the proj bias on host.
"""

import sys

sys.path.insert(0, "/opt/trn_rl_repo")

import numpy as np
import ml_dtypes

import concourse.bass as bass
import concourse.bacc as bacc_mod
import concourse.tile as tile
from concourse import mybir

F32 = mybir.dt.float32
BF16 = mybir.dt.bfloat16
F8E4 = mybir.dt.float8e4
U8 = mybir.dt.uint8

NPF8 = ml_dtypes.float8_e4m3
NPBF = ml_dtypes.bfloat16

EPS = 1e-5
NCORES = 8
N = 4096
SLICE = N // NCORES   # 512
B = 2
HEADS = 4
KD = 32
HD = 64
DIM = 256

AB = 32.0             # host scale on M (undone by exp's scale)
SHIFT = 2.0           # exp(S - SHIFT); cancels in softmax
SCH_A = float(8 * np.log2(np.e) / AB)
SCH_B = float(56.0 - 0.358 - SHIFT * 8 * np.log2(np.e))

# exp engine split per tile index 0..63 within a batch (4 tiles per jp):
# 1 -> ACT exact exp, 0 -> DVE schraudolph.
EXP_ACT = ([1 if (i * 21) // 32 != ((i + 1) * 21) // 32 else 0
            for i in range(32)] * 2)

_CACHE = {}


def _build_program():
    nc = bacc_mod.Bacc()

    x8_d = nc.dram_tensor("x8", [B, 8, 128, 4, 2, 128], F8E4,
                          kind="ExternalInput")
    x8o_d = nc.dram_tensor("x8o", [B, 2, 128, SLICE], F8E4,
                           kind="ExternalInput")
    xr_d = nc.dram_tensor("xr66", [B, 2, 128, 660], BF16, kind="ExternalInput")
    wm_d = nc.dram_tensor("wm8", [128, 4, 2, 2, 128], F8E4,
                          kind="ExternalInput")
    cpp_d = nc.dram_tensor("cpp", [128, 2, 4], F32, kind="ExternalInput")
    wva_d = nc.dram_tensor("wva8", [128, 2, 260], F8E4, kind="ExternalInput")
    wcv_d = nc.dram_tensor("wconv", [128, 2, 9, 2, 128], BF16,
                           kind="ExternalInput")
    wb9_d = nc.dram_tensor("wb9", [9, 2, 128], BF16, kind="ExternalInput")
    ind_d = nc.dram_tensor("ind9", [9, 512], BF16, kind="ExternalInput")
    wp_d = nc.dram_tensor("wp", [128, 2, 2, 128], BF16, kind="ExternalInput")
    bp_d = nc.dram_tensor("bp", [128, 2], F32, kind="ExternalInput")
    out_d = nc.dram_tensor("out", [B, 2, 128, SLICE], F32,
                           kind="ExternalOutput")

    DR = mybir.MatmulPerfMode.DoubleRow

    with tile.TileContext(nc) as tc:
        with (
            tc.tile_pool(name="singles", bufs=1) as singles,
            tc.tile_pool(name="xp", bufs=18) as xp,
            tc.tile_pool(name="xrp", bufs=2) as xrp,
            tc.tile_pool(name="pp", bufs=2) as pp,
            tc.tile_pool(name="vap", bufs=32) as vap,
            tc.tile_pool(name="ep", bufs=6) as ep,
            tc.tile_pool(name="rp", bufs=4) as rp,
            tc.tile_pool(name="zp", bufs=8) as zp,
            tc.tile_pool(name="op", bufs=4) as op_,
            tc.tile_pool(name="drp", bufs=4, space="DRAM") as drp,
            tc.tile_pool(name="spool", bufs=3, space="PSUM") as spool,
            tc.tile_pool(name="ypool", bufs=2, space="PSUM") as ypool,
        ):
            wm_s = singles.tile([128, 4, 2, 2, 128], F8E4, tag="wm")
            cpp_s = singles.tile([128, 2, 4], F32, tag="cpp")
            wva_s = singles.tile([128, 2, 260], F8E4, tag="wva")
            wcv_s = singles.tile([128, 2, 9, 2, 128], BF16, tag="wcv")
            wb9_s = singles.tile([9, 2, 128], BF16, tag="wb9")
            ind_s = singles.tile([9, 512], BF16, tag="ind")
            wp_s = singles.tile([128, 2, 2, 128], BF16, tag="wp")
            bp_s = singles.tile([128, 2], F32, tag="bp")
            ones_s = singles.tile([1, 128], F32, tag="ones")
            nc.vector.memset(ones_s, 1.0)
            nsh_s = singles.tile([128, 1], F32, tag="nsh")
            nc.vector.memset(nsh_s, -SHIFT)

            def load_weights():
                for t, d in [(wm_s, wm_d), (cpp_s, cpp_d), (wva_s, wva_d),
                             (wcv_s, wcv_d), (wb9_s, wb9_d), (ind_s, ind_d),
                             (wp_s, wp_d), (bp_s, bp_d)]:
                    nc.gpsimd.dma_start(t, d.ap())

            state = {}
            TAPS = [(dy, dx) for dy in (-1, 0, 1) for dx in (-1, 0, 1)]

            def load_x(b, qs=None):
                qs = qs or [nc.sync, nc.scalar]
                xo = xp.tile([128, 2, 512], F8E4, tag="x", name=f"xo{b}")
                qs[0].dma_start(
                    xo, x8o_d.ap()[b].rearrange("a p c -> p a c"))
                xts = []
                for n8 in range(8):
                    xt = xp.tile([128, 4, 2, 128], F8E4, tag="x",
                                 name=f"x{b}_{n8}")
                    qs[n8 % len(qs)].dma_start(xt, x8_d.ap()[b, n8])
                    xts.append(xt)
                xrt = xrp.tile([128, 2, 660], BF16, tag="xr", name=f"xr{b}")
                nc.gpsimd.dma_start(
                    xrt, xr_d.ap()[b].rearrange("a p c -> p a c"))
                state[b] = dict(xts=xts, xo=xo, xrt=xrt)

            def pgen(b, hs=range(HEADS)):
                st = state[b]
                xo = st["xo"]
                if "p8" not in st:
                    st["p8"] = pp.tile([128, 2, 4, 512], F8E4, tag="p",
                                       name=f"p{b}")
                    st["vats"] = [None] * 16
                p8 = st["p8"]
                for h in hs:
                    for mc in range(2):
                        ps = spool.tile([128, 512], F32, tag="s",
                                        name=f"pps{b}_{h}_{mc}")
                        nc.tensor.matmul(ps, wm_s[:, h, mc, :, :], xo,
                                         start=True, stop=True, perf_mode=DR)
                        nc.scalar.activation(
                            p8[:, mc, h, :], ps,
                            mybir.ActivationFunctionType.Identity,
                            bias=cpp_s[:, mc, h:h + 1])

            def vatgen(b, jps):
                st = state[b]
                xts = st["xts"]
                for jp in jps:
                    vt = vap.tile([128, 4, 2, 96], F8E4, tag="va",
                                  name=f"va{b}_{jp}")
                    nc.gpsimd.memset(vt[:, :, :, 64], 1.0)
                    ps = spool.tile([128, 1024], F32, tag="s",
                                    name=f"vps{b}_{jp}")
                    for jt in range(2):
                        n8, sub = divmod(2 * jp + jt, 4)
                        nc.tensor.matmul(
                            ps[:, jt * 512:jt * 512 + 260],
                            xts[n8][:, sub, :, :],
                            wva_s, start=True, stop=True, perf_mode=DR)
                    psv = ps.rearrange("p (t c) -> p t c", c=512)[:, :, 0:260] \
                        .rearrange("p t (h v) -> p t h v", v=65) \
                        .rearrange("p t h v -> p h t v")
                    nc.vector.tensor_scalar(
                        out=vt[:, :, :, 0:64], in0=psv[:, :, :, 0:64],
                        scalar1=1.0, scalar2=None, op0=mybir.AluOpType.mult)
                    st["vats"][jp] = vt

            def attn_pair(b, ph, inserts=None, tail=False):
                """S+exp+AV+norm for head pair ph of batch b."""
                st = state[b]
                xts, p8, vats = st["xts"], st["p8"], st["vats"]
                yts = [ypool.tile([128, 512], F32, tag="y",
                                  name=f"y{b}_{ph}_{hh}") for hh in range(2)]
                for jp in range(16):
                    for cb in (inserts or {}).get(jp, []):
                        cb()
                    et = ep.tile([128, 2, 1024], F8E4, tag="e",
                                 name=f"e{b}_{ph}_{jp}")
                    for par in range(2):
                        jc = 2 * jp + par
                        n8, sub = divmod(jc, 4)
                        sp = spool.tile([128, 1024], F32, tag="s",
                                        name=f"s{b}_{ph}_{jc}")
                        for hh in range(2):
                            h = 2 * ph + hh
                            nc.tensor.matmul(
                                sp[:, hh * 512:(hh + 1) * 512],
                                xts[n8][:, sub, :, :],
                                p8[:, :, h, :], start=True, stop=True,
                                perf_mode=DR)
                        idx = 2 * jp + par
                        if EXP_ACT[32 * (ph % 2) + idx]:
                            nc.scalar.activation(
                                et[:, par, :], sp,
                                mybir.ActivationFunctionType.Exp,
                                bias=nsh_s, scale=1.0 / AB)
                        else:
                            nc.vector.tensor_scalar(
                                out=et[:, par, :].bitcast(U8), in0=sp,
                                scalar1=SCH_A, scalar2=SCH_B,
                                op0=mybir.AluOpType.mult,
                                op1=mybir.AluOpType.add)
                    for hh in range(2):
                        h = 2 * ph + hh
                        nc.tensor.matmul(
                            yts[hh][0:65, :],
                            vats[jp][:, h, :, 0:65],
                            et[:, :, hh * 512:(hh + 1) * 512],
                            start=(jp == 0), stop=(jp == 15),
                            perf_mode=DR)
                # norm for this pair: copy y + recip (frees psum fast),
                # broadcast r, then zy with all-SBUF operands (2x mode)
                zy = zp.tile([128, 512], BF16, tag="zy", name=f"zy{b}_{ph}")
                yss, rts = [], []
                for hh in range(2):
                    ys = rp.tile([64, 512], F32, tag="ys",
                                 name=f"ys{b}_{ph}_{hh}")
                    nc.vector.tensor_copy(out=ys, in_=yts[hh][0:64, :])
                    rt = rp.tile([1, 512], F32, tag="r",
                                 name=f"r{b}_{ph}_{hh}")
                    nc.vector.reciprocal(rt, yts[hh][64:65, :])
                    yss.append(ys)
                    rts.append(rt)
                if tail:
                    for hh in range(2):
                        rb = spool.tile([128, 512], F32, tag="s",
                                        name=f"rbm{b}_{ph}_{hh}")
                        nc.tensor.matmul(rb, ones_s, rts[hh], start=True,
                                         stop=True)
                        nc.vector.tensor_tensor(
                            out=zy[64 * hh:64 * hh + 64, :],
                            in0=yss[hh], in1=rb[0:64, :],
                            op=mybir.AluOpType.mult)
                else:
                    rd = drp.tile([2, 512], F32, tag="rd",
                                  name=f"rd{b}_{ph}")
                    for hh in range(2):
                        nc.gpsimd.dma_start(rd[hh:hh + 1, :], rts[hh])
                    rbs = rp.tile([128, 2, 512], F32, tag="rb",
                                  name=f"rb{b}_{ph}")
                    nc.gpsimd.dma_start(
                        rbs, bass.AP(tensor=rd.tensor, offset=rd.offset,
                                     ap=[[0, 128]] + [list(p)
                                                      for p in rd.ap]))

                    for hh in range(2):
                        nc.gpsimd.tensor_tensor(
                            out=zy[64 * hh:64 * hh + 64, :],
                            in0=yss[hh], in1=rbs[0:64, hh, :],
                            op=mybir.AluOpType.mult)
                st.setdefault("zys", {})[ph] = zy

            def conv_part(b, mc, ts):
                st = state[b]
                xrv = st["xrt"].rearrange("p a (r c) -> p a r c", c=66)
                key = f"cps{mc}"
                if key not in st:
                    st[key] = spool.tile([128, 512], F32, tag="s",
                                         name=f"cps{b}_{mc}")
                ps = st[key]
                psv = ps.rearrange("p (r c) -> p r c", c=64)
                for t in ts:
                    dy, dx = TAPS[t]
                    for kc in range(2):
                        nc.tensor.matmul(
                            psv, wcv_s[:, kc, t, mc, :],
                            xrv[:, kc, 1 + dy:9 + dy, 1 + dx:65 + dx],
                            start=(t == 0 and kc == 0), stop=False)

            def conv_fin(b, mc):
                st = state[b]
                ps = st[f"cps{mc}"]
                nc.tensor.matmul(ps, wb9_s[:, mc, :], ind_s,
                                 start=False, stop=True)
                zt = zp.tile([128, 512], BF16, tag="z", name=f"z{b}_{mc}")
                nc.vector.tensor_scalar(
                    out=zt, in0=ps, scalar1=1.0, scalar2=None,
                    op0=mybir.AluOpType.mult)
                st.setdefault("zs", {})[mc] = zt

            def proj_early(b):
                st = state[b]
                st["jps"] = []
                for mo in range(2):
                    ps = spool.tile([128, 512], F32, tag="s",
                                    name=f"jps{b}_{mo}")
                    nc.tensor.matmul(ps, wp_s[:, 0, mo, :], st["zs"][0],
                                     start=True, stop=False)
                    nc.tensor.matmul(ps, wp_s[:, 1, mo, :], st["zs"][1],
                                     start=False, stop=False)
                    nc.tensor.matmul(ps, wp_s[:, 0, mo, :], st["zys"][0],
                                     start=False, stop=False)
                    st["jps"].append(ps)

            def proj_fin(b):
                st = state[b]
                for mo in range(2):
                    ps = st["jps"][mo]
                    nc.tensor.matmul(ps, wp_s[:, 1, mo, :], st["zys"][1],
                                     start=False, stop=True)
                    ot = op_.tile([128, 512], F32, tag="o", name=f"o{b}_{mo}")
                    nc.vector.tensor_scalar(
                        out=ot, in0=ps, scalar1=bp_s[:, mo:mo + 1],
                        scalar2=None, op0=mybir.AluOpType.add)
                    nc.sync.dma_start(out_d.ap()[b, mo], ot)

            def proj(b):
                proj_early(b)
                proj_fin(b)

            load_weights()
            load_x(0)
            pgen(0, [0, 1])
            vatgen(0, range(2))
            pgen(0, [2, 3])
            vatgen(0, range(2, 4))
            load_x(1, qs=[nc.gpsimd])
            # pair stream (b=0, ph=0): finish b0 vats
            insA = {1 + k: [lambda k=k: vatgen(0, [4 + k])] for k in range(12)}
            attn_pair(0, 0, inserts=insA)
            # pair stream (b=0, ph=1): b1 qkv; deferred zys of (0,0)
            insB = {1: [lambda: pgen(1, [0, 1])],
                    2: [lambda: pgen(1, [2, 3])]}
            for k in range(16):
                key = 3 + (k * 12) // 16
                insB[key] = insB.get(key, []) + [lambda k=k: vatgen(1, [k])]
            attn_pair(0, 1, inserts=insB)
            # pair stream (b=1, ph=0): conv(b0) chunks + proj(0)
            work = [[0, 1], [2, 3], [4, 5], [6, 7], [8]]

            def conv_sched(b):
                cbs = []
                for mc in range(2):
                    for ts in work:
                        cbs.append(lambda mc=mc, ts=ts: conv_part(b, mc, ts))
                    cbs.append(lambda mc=mc: conv_fin(b, mc))
                return cbs

            insC = {1 + k: [cb] for k, cb in enumerate(conv_sched(0))}
            insC[13] = [lambda: proj(0)]
            attn_pair(1, 0, inserts=insC)
            # pair stream (b=1, ph=1): conv(b1) chunks
            insD = {1 + k: [cb] for k, cb in enumerate(conv_sched(1))}
            attn_pair(1, 1, inserts=insD, tail=True)
            proj(1)

    nc.compile()
    return nc


def _prep_maps(x, qkv_w, qkv_g, qkv_b, qkv_m, qkv_v,
               proj_w, proj_g, proj_b, proj_m, proj_v,
               pe_w, pe_g, pe_b, pe_m, pe_v):
    f = np.float32
    H = W = 64

    s_qkv = (qkv_g / np.sqrt(qkv_v + EPS)).astype(f)
    t_qkv = (qkv_b - qkv_m * s_qkv).astype(f)
    Wfull = (qkv_w[:, :, 0, 0] * s_qkv[:, None]).astype(f)   # [512, 256]

    hs = np.arange(HEADS)
    qrows = (128 * hs[:, None] + np.arange(KD)[None, :]).ravel()
    krows = qrows + KD
    vrows = (128 * hs[:, None] + 2 * KD + np.arange(HD)[None, :]).ravel()
    scale = f(KD) ** -0.5

    Wq = Wfull[qrows]; bq = t_qkv[qrows]
    Wk = Wfull[krows]
    Wv = Wfull[vrows]; bv = t_qkv[vrows]

    # M_h[c,d] = scale * (Wk_h^T Wq_h)[c,d]; c_h = scale * Wk_h^T bq_h
    Ms = np.stack([scale * Wk[32 * h:32 * h + 32].T @ Wq[32 * h:32 * h + 32]
                   for h in range(HEADS)])            # [4, 256(c), 256(d)]
    cs = np.stack([scale * Wk[32 * h:32 * h + 32].T @ bq[32 * h:32 * h + 32]
                   for h in range(HEADS)])            # [4, 256]

    # wm8[d_part, h, mc, kc, c2] = AB * Ms[h, mc*128+c2, kc*128+d]
    wm8 = np.ascontiguousarray(
        (AB * Ms).reshape(4, 2, 128, 2, 128)          # [h, mc, c2, kc, d]
        .transpose(4, 0, 1, 3, 2)).astype(NPF8)
    cpp = np.ascontiguousarray(
        (AB * cs).reshape(4, 2, 128).transpose(2, 1, 0)).astype(f)

    # wva8[c_part, kc, 65h+d] = Wv[64h+d, kc*128+c]; col 65h+64 = 0 (ones)
    Wva = np.zeros((DIM, 260), f)
    for h in range(HEADS):
        Wva[:, 65 * h:65 * h + 64] = Wv[64 * h:64 * h + 64].T
    wva8 = np.ascontiguousarray(
        Wva.reshape(2, 128, 260).transpose(1, 0, 2)).astype(NPF8)

    s_pe = (pe_g / np.sqrt(pe_v + EPS)).astype(f)
    t_pe = (pe_b - pe_m * s_pe).astype(f)
    w9 = (pe_w[:, 0].reshape(DIM, 9) * s_pe[:, None]).astype(f)

    # wconv[c, kc, t, mc, o] = w9[mc*128+o, t] * Wv[mc*128+o, kc*128+c]
    Wt = w9.T[:, :, None] * Wv[None, :, :]            # [9, 256(o), 256(c)]
    wconv = np.ascontiguousarray(
        Wt.reshape(9, 2, 128, 2, 128)                 # [t, mc, o, kc, c]
        .transpose(4, 3, 0, 1, 2)).astype(NPBF)
    # wb9[t, mc, o] = bv[mc*128+o] * w9[mc*128+o, t]
    wb9 = np.ascontiguousarray(
        (bv[:, None] * w9).reshape(2, 128, 9).transpose(2, 0, 1)).astype(NPBF)

    s_p = (proj_g / np.sqrt(proj_v + EPS)).astype(f)
    t_p = (proj_b - proj_m * s_p).astype(f)
    P_eff = (proj_w[:, :, 0, 0] * s_p[:, None]).astype(f)
    bp_full = (t_p + P_eff @ t_pe + P_eff @ bv).astype(f)
    wp = np.ascontiguousarray(
        P_eff.reshape(2, 128, 2, 128).transpose(3, 2, 0, 1)).astype(NPBF)
    bp = np.ascontiguousarray(bp_full.reshape(2, 128).T).astype(f)

    xf = np.ascontiguousarray(x.reshape(B, DIM, N)).astype(f)
    # x8[b, n8, c_part, sub, kc, j] = x[b, kc*128+c_part, n8*512+sub*128+j]
    x8 = np.ascontiguousarray(
        xf.reshape(B, 2, 128, 8, 4, 128)
        .transpose(0, 3, 2, 4, 1, 5)).astype(NPF8)
    x8kc = np.ascontiguousarray(xf.reshape(B, 2, 128, N)).astype(NPF8)

    shared = dict(x8=x8, wm8=wm8, cpp=cpp, wva8=wva8, wconv=wconv,
                  wb9=wb9, wp=wp, bp=bp)

    in_maps = []
    for c in range(NCORES):
        i0 = SLICE * c
        r0 = i0 // W
        m = dict(shared)
        m["x8o"] = np.ascontiguousarray(x8kc[:, :, :, i0:i0 + SLICE])
        xr66 = np.zeros((B, DIM, 10, 66), f)
        for rr in range(10):
            gr = r0 - 1 + rr
            if 0 <= gr < H:
                xr66[:, :, rr, 1:65] = xf[:, :, gr * W:(gr + 1) * W]
        m["xr66"] = np.ascontiguousarray(
            xr66.reshape(B, 2, 128, 660)).astype(NPBF)
        ins = np.zeros((9, 512), f)
        TAPS = [(dy, dx) for dy in (-1, 0, 1) for dx in (-1, 0, 1)]
        iv = ins.reshape(9, 8, 64)
        for t, (dy, dx) in enumerate(TAPS):
            for rr in range(8):
                if 0 <= r0 + rr + dy < H:
                    iv[t, rr, max(0, -dx):64 - max(0, dx)] = 1.0
        m["ind9"] = ins.astype(NPBF)
        in_maps.append(m)
    return in_maps


def _gather(results):
    full = np.zeros((B, DIM, N), np.float32)
    for c, res in enumerate(results):
        o = res["out"]
        for mo in range(2):
            full[:, 128 * mo:128 * (mo + 1),
                 SLICE * c:SLICE * (c + 1)] = o[:, mo]
    return full.reshape(B, DIM, 64, 64)


def run(inputs, trace=False, trace_kwargs=None):
    from concourse.bass_utils import run_bass_kernel_spmd
    if "nc" not in _CACHE:
        _CACHE["nc"] = _build_program()
    nc = _CACHE["nc"]
    in_maps = _prep_maps(**inputs)
    res = run_bass_kernel_spmd(
        nc, in_maps, core_ids=list(range(NCORES)),
        trace=trace, **(trace_kwargs or {}))
    return _gather(res.results), res


def kernel(**inputs):
    inputs = {k: np.asarray(v) for k, v in inputs.items()}
    out, _ = run(inputs, trace=False)
    return out
